# revision 12
# baseline (speedup 1.0000x reference)
"""Causal self-attention with RoPE on 8 Trainium2 NeuronCores.

Sharding: tensor-parallel over heads (2 heads/core) through QKV projection,
RoPE and attention; AllToAll reshards attention output from head-split to
token-split; out-projection is token-parallel with full out_w per core
(no reduction needed). Output: each core produces its 512-token slice.

Layouts (per core, f = feature, t = token, d = contraction):
  xT   [D, NT]    input transposed (d on partitions) - rhs/lhsT for projections
  q/k  [128, NT]  per-head, head-dim on partitions ("qT"): proj out [f, t]
  v    [NT, 256]  token-major: proj out [t, f]
  sT   [j, r]     scores transposed: lhsT=kT-tile, rhs=qT-block
  pT   [j, r]     exp(scores*scale) bf16
  oT   [dv, r]    PV: lhsT=v-tile [j, dv], rhs=pT [j, r]
  denom[1, r]     ones-matmul over dacc (fp32 accumulated pT)
  out  [t, e]     out-proj: lhsT=attnT-tile [dv, t], rhs=out_wT [dv, e]
"""
import math
import numpy as np
import ml_dtypes

import concourse.bass as bass
import concourse.mybir as mybir
import concourse.tile as tile
from concourse import bacc
from concourse.bass_utils import run_bass_kernel_spmd

F32 = mybir.dt.float32
F32R = mybir.dt.float32r
BF16 = mybir.dt.bfloat16
AF = mybir.ActivationFunctionType
ALU = mybir.AluOpType

N_CORES = 8


def legalize_waits(nc, max_waits=1):
    """This walrus build only encodes one sync-wait per TPB instruction.
    Move extra waits emitted by Tile onto same-engine NoOps inserted
    immediately before the instruction."""
    n_split = 0
    for fn in nc.m.functions:
        for bb in fn.blocks:
            new_insts = []
            for inst in bb.instructions:
                si = getattr(inst, "sync_info", None)
                waits = list(si.on_wait) if si is not None and si.on_wait else []
                if len(waits) > max_waits and type(inst).__name__ != "InstNoOp":
                    extra, keep = waits[:-max_waits], waits[-max_waits:]
                    for k, w in enumerate(extra):
                        nop = mybir.InstNoOp(
                            name=f"{inst.name}_waitnop{k}",
                            engine=inst.engine,
                            ins=[],
                            outs=[],
                            sync_info=mybir.SyncInfo(on_wait=[w], on_update=[]),
                        )
                        nc.register_instruction(nop)
                        new_insts.append(nop)
                    inst.sync_info = mybir.SyncInfo(
                        on_wait=keep, on_update=list(si.on_update)
                    )
                    n_split += 1
                new_insts.append(inst)
            bb.instructions = new_insts
    return n_split


def build_nc(B=2, T=2048, D=2048, H=16, fake_cc=False):
    HD = D // H                  # 128, head dim
    NT = B * T                   # total tokens
    HPC = H // N_CORES           # heads per core (2)
    DC = HPC * HD                # head channels per core (256)
    KT = D // 128                # contraction tiles for projections (16)
    NB = NT // 512               # 512-token blocks overall (8)
    RB = T // 512                # 512-token blocks per batch element (4)
    S = NT // N_CORES            # AllToAll shard = tokens per core (512)
    EB = D // 512                # 512-wide out-feature blocks (4)
    SCALE = 1.0 / math.sqrt(HD)

    nc = bacc.Bacc("TRN2", target_bir_lowering=False, debug=False, num_devices=N_CORES)
    xT_e = nc.dram_tensor("xT", [D, NT], BF16, kind="ExternalInput")
    wqk_e = nc.dram_tensor("wqk", [D, 4 * HD], BF16, kind="ExternalInput")
    bqk_e = nc.dram_tensor("bqk", [4 * HD], F32, kind="ExternalInput")
    wv_e = nc.dram_tensor("wv", [D, DC], BF16, kind="ExternalInput")
    bv_e = nc.dram_tensor("bv", [DC], F32, kind="ExternalInput")
    cos_e = nc.dram_tensor("cosT", [HD, NT], BF16, kind="ExternalInput")
    sin_e = nc.dram_tensor("sinT", [HD, NT], BF16, kind="ExternalInput")
    masks_e = nc.dram_tensor("masks", [4, 128, 512], BF16, kind="ExternalInput")
    owT_e = nc.dram_tensor("owT", [D, D], BF16, kind="ExternalInput")
    ob_e = nc.dram_tensor("ob", [D], F32, kind="ExternalInput")
    out_e = nc.dram_tensor("out", [S, D], F32, kind="ExternalOutput")

    with tile.TileContext(nc) as tc:
        with tc.tile_pool(name="persist", bufs=1) as pp, \
             tc.tile_pool(name="dram", bufs=1, space="DRAM") as dp:
            # ---- persistent tiles ----
            qk = [pp.tile([128, NT], BF16, tag=f"qk{m}", name=f"qk{m}") for m in range(4)]
            v_sb = pp.tile([128, NT // 128, DC], BF16, tag="v", name="v")
            masks = pp.tile([128, 4, 512], BF16, tag="masks", name="masks")
            nc.sync.dma_start(masks[:], masks_e.rearrange("m p c -> p m c"))
            bqk = pp.tile([128, 4], F32, tag="bqk", name="bqk")
            nc.sync.dma_start(bqk[:], bqk_e.rearrange("(m p) -> p m", p=128))
            bv1 = pp.tile([1, DC], F32, tag="bv1", name="bv1")
            nc.sync.dma_start(bv1[:], bv_e[None, :])
            bvB = pp.tile([128, DC], F32, tag="bvB", name="bvB")
            nc.gpsimd.partition_broadcast(bvB[:], bv1[:])
            ob1 = pp.tile([1, D], F32, tag="ob1", name="ob1")
            nc.sync.dma_start(ob1[:], ob_e[None, :])
            obB = pp.tile([128, D], F32, tag="obB", name="obB")
            nc.gpsimd.partition_broadcast(obB[:], ob1[:])
            ones = pp.tile([128, 1], BF16, tag="ones", name="ones")
            nc.vector.memset(ones[:], 1.0)

            Zs = [dp.tile([N_CORES, HD, S], BF16, tag=f"Z{i}", name=f"Z{i}")
                  for i in range(HPC)]
            ZGs = [dp.tile([N_CORES, HD, S], BF16, tag=f"ZG{i}", name=f"ZG{i}")
                   for i in range(HPC)]

            # ---- phase 1: QKV projections + RoPE ----
            with tc.tile_pool(name="p1w", bufs=1) as wp, \
                 tc.tile_pool(name="p1x", bufs=3) as xp, \
                 tc.tile_pool(name="p1t", bufs=3) as tp, \
                 tc.tile_pool(name="p1ps", bufs=6, space="PSUM") as ps:
                wqk_sb = wp.tile([128, KT, 4 * HD], BF16, tag="wqk", name="wqk")
                nc.scalar.dma_start(wqk_sb[:], wqk_e.rearrange("(kt p) f -> p kt f", p=128))
                wv_sb = wp.tile([128, KT, DC], BF16, tag="wv", name="wv")
                nc.scalar.dma_start(wv_sb[:], wv_e.rearrange("(kt p) f -> p kt f", p=128))
                cos_sb = wp.tile([128, NT], BF16, tag="cos", name="cos")
                nc.scalar.dma_start(cos_sb[:], cos_e[:])
                sin_sb = wp.tile([128, NT], BF16, tag="sin", name="sin")
                nc.scalar.dma_start(sin_sb[:], sin_e[:])

                for blk in range(NB):
                    tsl = bass.ts(blk, 512)
                    xb = xp.tile([128, KT, 512], BF16, tag="xb", name="xb")
                    xTr = xT_e.rearrange("(kt p) t -> p kt t", p=128)
                    nc.sync.dma_start(xb[:, :KT // 2, :], xTr[:, :KT // 2, tsl])
                    nc.sync.dma_start(xb[:, KT // 2:, :], xTr[:, KT // 2:, tsl])
                    # q/k projections -> [f, t], f-tile m: q0 q1 k0 k1
                    for m in range(4):
                        psqk = ps.tile([128, 512], F32, tag="ps", name="ps")
                        for kt in range(KT):
                            nc.tensor.matmul(
                                psqk[:],
                                wqk_sb[:, kt, bass.ts(m, 128)],
                                xb[:, kt, :],
                                start=(kt == 0),
                                stop=(kt == KT - 1),
                            )
                        # evict with bias on ACT
                        nc.scalar.activation(
                            qk[m][:, tsl], psqk[:], AF.Identity,
                            bias=bqk[:, m:m + 1], scale=1.0,
                        )
                        # RoPE in place: qk = qk*cos + swap(qk)*s2, where
                        # s2 = sin with first half negated (host-prepared) and
                        # swap exchanges partition halves (engines cannot read
                        # across partitions -> use SBUF->SBUF DMA).
                        qm = qk[m][:, tsl]
                        qsw = tp.tile([128, 512], BF16, tag="qsw", name="qsw")
                        nc.sync.dma_start(qsw[0:64, :], qm[64:128, :])
                        nc.sync.dma_start(qsw[64:128, :], qm[0:64, :])
                        nc.vector.tensor_mul(qsw[:], qsw[:], sin_sb[:, tsl])
                        nc.vector.tensor_mul(qm, qm, cos_sb[:, tsl])
                        nc.vector.tensor_add(qm, qm, qsw[:])
                    # v projection -> [t, f]
                    for tt in range(4):
                        psv = ps.tile([128, 512], F32, tag="ps", name="ps")
                        for kt in range(KT):
                            nc.tensor.matmul(
                                psv[:, :DC],
                                xb[:, kt, bass.ts(tt, 128)],
                                wv_sb[:, kt, :],
                                start=(kt == 0),
                                stop=(kt == KT - 1),
                            )
                        nc.vector.tensor_add(v_sb[:, blk * 4 + tt, :], psv[:, :DC], bvB[:])

            # ---- phase 2: attention per (head, batch) ----
            # 1024-wide score chunks (2 key-tiles per exp) amortize ACT
            # overhead. The softmax denominator is accumulated on the PE via
            # a ones-vector matmul per chunk half (PSUM accumulation across
            # the whole row block) - no elementwise accumulation needed.
            # hh-outer so each head-half's AllToAll fires while the other
            # half computes.
            with tc.tile_pool(name="p2t", bufs=4) as tp2, \
                 tc.tile_pool(name="p2ps", bufs=2, space="PSUM") as ps2, \
                 tc.tile_pool(name="p2po", bufs=2, space="PSUM") as ps2o, \
                 tc.tile_pool(name="p2pd", bufs=2, space="PSUM") as ps2d:
                for hh in range(HPC):
                    for b in range(B):
                        qT = qk[hh]
                        kTt = qk[2 + hh]
                        for rb in range(RB):
                            rsl = bass.ds(b * T + rb * 512, 512)
                            pso = ps2o.tile([128, 512], F32, tag="pso", name="pso")
                            psd = ps2d.tile([128, 512], F32, tag="psd", name="psd")
                            njt = 4 * rb + 4
                            for jc in range(njt // 2):
                                pss = ps2.tile([128, 1024], F32, tag="pss", name="pss")
                                for half in range(2):
                                    jt = 2 * jc + half
                                    nc.tensor.matmul(
                                        pss[:, bass.ts(half, 512)],
                                        kTt[:, bass.ds(b * T + jt * 128, 128)],
                                        qT[:, rsl],
                                        start=True, stop=True,
                                    )
                                pT = tp2.tile([128, 1024], BF16, tag="pT", name="pT")
                                nc.scalar.activation(pT[:], pss[:], AF.Exp, scale=SCALE)
                                m = 2 * jc - 4 * rb
                                if m >= 0:
                                    nc.vector.tensor_mul(
                                        pT[:], pT[:],
                                        masks[:, m:m + 2, :].rearrange("p a b -> p (a b)"),
                                    )
                                for half in range(2):
                                    jt = 2 * jc + half
                                    nc.tensor.matmul(
                                        psd[:1, :], ones[:], pT[:, bass.ts(half, 512)],
                                        start=(jt == 0), stop=(jt == njt - 1),
                                    )
                                    nc.tensor.matmul(
                                        pso[:],
                                        v_sb[:, (b * T) // 128 + jt, bass.ts(hh, HD)],
                                        pT[:, bass.ts(half, 512)],
                                        start=(jt == 0),
                                        stop=(jt == njt - 1),
                                    )
                            recip = tp2.tile([1, 512], F32, tag="recip", name="recip")
                            nc.vector.reciprocal(recip[:], psd[:1, :])
                            recipB = tp2.tile([128, 512], F32, tag="recipB", name="recipB")
                            nc.gpsimd.partition_broadcast(recipB[:], recip[:])
                            oT = tp2.tile([128, 512], BF16, tag="oT", name="oT")
                            nc.vector.tensor_mul(oT[:], pso[:], recipB[:])
                            g = b * RB + rb
                            for off in range(0, 512, S):
                                sh = (g * 512 + off) // S
                                nc.sync.dma_start(
                                    Zs[hh][sh, :, bass.ds((g * 512 + off) % S, min(512, S))],
                                    oT[:, bass.ds(off, min(512, S))],
                                )
                    # reshard this head-half while the next one computes
                    if fake_cc:
                        nc.sync.dma_start(ZGs[hh][:], Zs[hh][:])
                    else:
                        nc.gpsimd.collective_compute(
                            "AllToAll", ALU.bypass,
                            replica_groups=[list(range(N_CORES))],
                            ins=[Zs[hh][:]], outs=[ZGs[hh][:]],
                        )

            # ---- phase 4: out projection on own token slice ----
            with tc.tile_pool(name="p4z", bufs=1) as zp, \
                 tc.tile_pool(name="p4w", bufs=3) as owp, \
                 tc.tile_pool(name="p4t", bufs=3) as tp4, \
                 tc.tile_pool(name="p4ps", bufs=4, space="PSUM") as ps4:
                zgs = []
                for i in range(HPC):
                    zg = zp.tile([128, N_CORES, S], BF16, tag=f"zg{i}", name=f"zg{i}")
                    nc.sync.dma_start(
                        zg[:], ZGs[i][:].rearrange("c d s -> d c s")
                    )
                    zgs.append(zg)
                for e in range(EB):
                    ow = owp.tile([128, KT, 512], BF16, tag="ow", name="ow")
                    owr = owT_e.rearrange("(kt p) f -> p kt f", p=128)[:, :, bass.ts(e, 512)]
                    nc.scalar.dma_start(ow[:, :KT // 2, :], owr[:, :KT // 2, :])
                    nc.scalar.dma_start(ow[:, KT // 2:, :], owr[:, KT // 2:, :])
                    for tt in range(S // 128):
                        pso4 = ps4.tile([128, 512], F32, tag="ps4", name="ps4")
                        # zg0's contraction tiles first: out-proj starts after
                        # the first AllToAll, hiding the second one.
                        for ki in range(KT):
                            hhz, zt = ki // (KT // HPC), ki % (KT // HPC)
                            nc.tensor.matmul(
                                pso4[:],
                                zgs[hhz][:, zt, bass.ts(tt, 128)],
                                ow[:, HPC * zt + hhz, :],
                                start=(ki == 0),
                                stop=(ki == KT - 1),
                            )
                        of = tp4.tile([128, 512], F32, tag="of", name="of")
                        nc.vector.tensor_add(of[:], pso4[:], obB[:, bass.ts(e, 512)])
                        nc.sync.dma_start(out_e[bass.ts(tt, 128), bass.ts(e, 512)], of[:])

    nc.compile()          # Bacc pass pipeline (library loads, nop fusion, regs)
    legalize_waits(nc)    # must run after all nop-fusion passes
    bass.Bass.finalize(nc)  # freeze without re-running Bacc compile
    return nc


def _prep_inputs(x, rope_cos, rope_sin, qkv_w, qkv_b, out_w, out_b, B, T, D, H):
    HD = D // H
    NT = B * T
    HPC = H // N_CORES
    bf = ml_dtypes.bfloat16

    x2 = np.ascontiguousarray(x.reshape(NT, D).T).astype(bf)           # [D, NT]
    cosT = np.ascontiguousarray(
        np.tile(rope_cos[0, 0].T, (1, B))).astype(bf)                   # [HD, NT]
    s2 = np.tile(rope_sin[0, 0].T, (1, B)).copy()
    s2[:HD // 2] *= -1.0
    sinT = np.ascontiguousarray(s2).astype(bf)
    owT = np.ascontiguousarray(out_w.T).astype(bf)                      # [D, D]
    ob = out_b.astype(np.float32)

    c_grid = np.arange(512)[None, :]
    p_grid = np.arange(128)[:, None]
    masks = np.stack(
        [(c_grid >= 128 * m + p_grid) for m in range(4)]
    ).astype(bf)                                                        # [4,128,512]

    in_maps = []
    for c in range(N_CORES):
        heads = [HPC * c + i for i in range(HPC)]
        q_rows = np.concatenate([qkv_w[h * HD:(h + 1) * HD] for h in heads])
        k_rows = np.concatenate([qkv_w[D + h * HD:D + (h + 1) * HD] for h in heads])
        v_rows = np.concatenate([qkv_w[2 * D + h * HD:2 * D + (h + 1) * HD] for h in heads])
        wqk = np.ascontiguousarray(np.concatenate([q_rows, k_rows]).T).astype(bf)
        wv = np.ascontiguousarray(v_rows.T).astype(bf)
        bq = np.concatenate([qkv_b[h * HD:(h + 1) * HD] for h in heads])
        bk = np.concatenate([qkv_b[D + h * HD:D + (h + 1) * HD] for h in heads])
        bqk = np.concatenate([bq, bk]).astype(np.float32)
        bv = np.concatenate(
            [qkv_b[2 * D + h * HD:2 * D + (h + 1) * HD] for h in heads]
        ).astype(np.float32)
        in_maps.append({
            "xT": x2, "wqk": wqk, "bqk": bqk, "wv": wv, "bv": bv,
            "cosT": cosT, "sinT": sinT, "masks": masks,
            "owT": owT, "ob": ob,
        })
    return in_maps


_NC_CACHE = {}


def kernel(x, rope_cos, rope_sin, qkv_w, qkv_b, out_w, out_b):
    B, T, D = x.shape
    H = 16
    NT = B * T
    S = NT // N_CORES
    key = (B, T, D, H)
    if key not in _NC_CACHE:
        _NC_CACHE[key] = build_nc(B, T, D, H)
    nc = _NC_CACHE[key]
    in_maps = _prep_inputs(
        np.asarray(x), np.asarray(rope_cos), np.asarray(rope_sin),
        np.asarray(qkv_w), np.asarray(qkv_b), np.asarray(out_w),
        np.asarray(out_b), B, T, D, H,
    )
    res = run_bass_kernel_spmd(nc, in_maps, core_ids=list(range(N_CORES)))
    out = np.empty((NT, D), np.float32)
    for c in range(N_CORES):
        out[c * S:(c + 1) * S] = res.results[c]["out"]
    return out.reshape(B, T, D)
